# revision 12
# baseline (speedup 1.0000x reference)
"""Causal self-attention Trainium2 kernel (8 NeuronCores).

Sharding: tensor-parallel over heads x data-parallel over batch.
Core c handles batch b = c // 4 and head group g = c % 4 (4 heads of 16).
Each core computes q/k/v projections for its heads, causal attention, and a
partial output projection (its 256 columns of the 1024-wide contraction);
the host sums the 4 partials per batch.

v3 design (bf16, PE row-tiled scores, software-pipelined + work injection):
  - All matmul inputs bf16 (PSUM stays fp32). PE streams 1 col/cycle@2.4GHz.
  - Scores for a head pair are row-tiled: both heads' hd-dims occupy
    disjoint 64-partition bands of one qkT chunk, so the two K=64 matmuls
    share one moving stream and run concurrently (explicit tile_position).
  - Attention per head-pair pass: one 2-bank PSUM slot per round, one
    batched exp ACTIVATE over both heads ((N+352)cyc amortized), causal
    column trim on diagonal chunks, [128,128] triangle mask on DVE.
  - PV uses the ones-column denominator trick (M=65); normalization is
    fused into the PV drain (psum * bcast(1/den) -> bf16 yT).
  - QKV(b+1) and proj(b) are split into ~1us PE units and injected into
    the attention rounds (one unit after each exp emission) so the PE
    works under the ACT/PE-round shadow instead of serializing phases.
  - Host pre-swizzles inputs to [128, ...] partition-major layouts so all
    input DMAs move 4-8KB contiguous lines per partition.
"""

import numpy as np
import ml_dtypes

import concourse.bass as bass
from concourse import bacc
import concourse.mybir as mybir
import concourse.tile as tile
from concourse.bass_utils import run_bass_kernel_spmd

B, T, D, H = 2, 2048, 1024, 16
HD = D // H          # 64
HPC = 4              # heads per core
NCORES = 8
EQK = 2 * HPC * HD   # 512 rows of q+k per core
EV = HPC * HD        # 256 rows of v per core
TB = 512             # t/q block
NTB = T // TB        # 4
TC = 128             # t chunk
NTC = T // TC        # 16
DCH = D // 128       # 8 contraction chunks
F32 = mybir.dt.float32
BF16 = mybir.dt.bfloat16

_cache = {}


def _ensure_ntff_hook():
    """The agent image's antenv lacks axon_hooks; fabricate it so
    run_bass_kernel_spmd(trace=True) can capture NTFF profiles."""
    import sys
    import types
    try:
        import antenv.axon_hooks  # noqa: F401
        return
    except ImportError:
        pass
    try:
        import antenv
        from trn_agent_boot.trn_boot import _ntff_profile_via_ctypes
        hook = {"h": _ntff_profile_via_ctypes("/opt/axon/libaxon_pjrt.so")}
        m = types.ModuleType("antenv.axon_hooks")
        m.get_axon_ntff_profile_hook = lambda: hook["h"]
        m.set_axon_ntff_profile_hook = lambda h: hook.update(h=h)
        sys.modules["antenv.axon_hooks"] = m
        antenv.axon_hooks = m
    except Exception:
        pass


def _build_nc():
    nc = bacc.Bacc("TRN2", target_bir_lowering=False, debug=False,
                  num_devices=NCORES)
    # host-swizzled partition-major layouts (fat contiguous DMA lines)
    xs = nc.dram_tensor("xs", [NTB * 128, DCH, TB], BF16, kind="ExternalInput")
    wqk = nc.dram_tensor("wqk", [128, DCH, EQK], BF16, kind="ExternalInput")
    wv = nc.dram_tensor("wv", [128, DCH, EV], BF16, kind="ExternalInput")
    wp = nc.dram_tensor("wp", [128, 2, D], BF16, kind="ExternalInput")
    tri = nc.dram_tensor("tri", [128, 128], BF16, kind="ExternalInput")
    out = nc.dram_tensor("out", [T, D], F32, kind="ExternalOutput")

    with tile.TileContext(nc) as tc:
        with (
            nc.allow_low_precision(reason="bf16 matmul inputs; psum stays fp32"),
            tc.tile_pool(name="persist", bufs=1) as persist,
            tc.tile_pool(name="xin", bufs=2) as xin,
            tc.tile_pool(name="work", bufs=4) as work,
            tc.tile_pool(name="probsp", bufs=4) as probsp,
            tc.tile_pool(name="outp", bufs=3) as outp,
            tc.tile_pool(name="ps_mm", bufs=2, space="PSUM") as ps_mm,
            tc.tile_pool(name="ps_pv", bufs=4, space="PSUM") as ps_pv,
        ):
            # ---- input DMAs: x(0) first (QKV(0) is the critical path) ----
            x_tiles = {}
            x_tiles[0] = xin.tile([128, DCH, TB], BF16, tag="x", name="x_t0")
            nc.sync.dma_start(x_tiles[0][:], xs[0:128, :, :])
            wqk_sb = persist.tile([128, DCH, EQK], BF16)   # 8KB/part
            nc.sync.dma_start(wqk_sb[:], wqk[:, :, :])
            wv_sb = persist.tile([128, DCH, EV], BF16)     # 4KB/part
            nc.sync.dma_start(wv_sb[:], wv[:, :, :])
            wp_sb = persist.tile([128, 2, D], BF16)        # 4KB/part
            nc.sync.dma_start(wp_sb[:], wp[:, :, :])
            tri_sb = persist.tile([128, 128], BF16)
            nc.sync.dma_start(tri_sb[:], tri[:, :])

            # qkT[p, c, t]: c in {0: q h01, 1: q h23, 2: k h01, 3: k h23};
            # head pair member at base partition 0/64.
            qkT = persist.tile([128, 4, T], BF16, name="qkT")    # 16KB/part
            # v_sb[t_chunk]: [128, h, 65]; col 64 of each head slot is 1.0
            v_sb = [persist.tile([128, HPC, HD + 1], BF16, tag=f"v{i}",
                                name=f"v{i}")
                    for i in range(NTC)]
            # yT: normalized attention output, [p, c, t]; c=0 heads01, 1 h23
            yT = persist.tile([128, 2, T], BF16, name="yT")      # 8KB/part

            # ones columns for the denominator trick
            for i in range(NTC):
                nc.gpsimd.memset(v_sb[i][:, :, HD], 1.0)

            # ---------- PE work units (for injection) ----------
            def x_dma(b):
                x_tiles[b] = xin.tile([128, DCH, TB], BF16, tag="x",
                                      name=f"x_t{b}")
                nc.sync.dma_start(x_tiles[b][:],
                                  xs[128 * b:128 * (b + 1), :, :])

            def unit_qk(b, pr):
                def go():
                    x_t = x_tiles[b]
                    ps = ps_mm.tile([128, 2, TB], F32, tag="mm", name="ps_qk")
                    for sub in range(2):
                        ec = 2 * pr + sub
                        for dc in range(DCH):
                            nc.tensor.matmul(
                                ps[:, sub, :],
                                wqk_sb[:, dc, 128 * ec:128 * (ec + 1)],
                                x_t[:, dc, :],
                                start=(dc == 0), stop=(dc == DCH - 1))
                    nc.vector.tensor_copy(
                        qkT[:, 2 * pr:2 * pr + 2, b * TB:(b + 1) * TB], ps[:])
                return go

            def unit_v(b, pr):
                def go():
                    x_t = x_tiles[b]
                    ps = ps_mm.tile([128, 2, TB], F32, tag="mm", name="ps_v")
                    for sub in range(2):
                        t2 = 2 * pr + sub
                        for dc in range(DCH):
                            nc.tensor.matmul(
                                ps[:, sub, 0:EV],
                                x_t[:, dc, 128 * t2:128 * (t2 + 1)],
                                wv_sb[:, dc, :],
                                start=(dc == 0), stop=(dc == DCH - 1))
                    for sub in range(2):
                        tc_i = 4 * b + 2 * pr + sub
                        nc.vector.tensor_copy(
                            v_sb[tc_i][:, :, 0:HD],
                            ps[:, sub, 0:EV].rearrange("p (h f) -> p h f",
                                                       h=HPC))
                return go

            def unit_proj(b, tq):
                def go():
                    tc_i = 4 * b + tq
                    ps = ps_mm.tile([128, 2, TB], F32, tag="mm", name="ps_pj")
                    for e in range(2):
                        for c in range(2):
                            nc.tensor.matmul(
                                ps[:, e, :],
                                yT[:, c, 128 * tc_i:128 * (tc_i + 1)],
                                wp_sb[:, c, 512 * e:512 * (e + 1)],
                                start=(c == 0), stop=(c == 1))
                    o_sb = outp.tile([128, 2, TB], F32, tag="o")
                    nc.vector.tensor_copy(o_sb[:], ps[:])
                    nc.sync.dma_start(
                        out[128 * tc_i:128 * (tc_i + 1), :],
                        o_sb.rearrange("p a b -> p (a b)"))
                return go

            pending = []

            def inject(k=1):
                for _ in range(k):
                    if pending:
                        pending.pop(0)()

            # ---------- attention ----------
            def emit_scores(b, pr, kc):
                j = kc - 4 * b
                q0 = 128 * j if j >= 0 else 0
                ps = ps_mm.tile([128, 2, TB], F32, tag="mm", name="ps_s")
                for sub in range(2):
                    nc.tensor.matmul(
                        ps[:, sub, q0:],
                        qkT[64 * sub:64 * sub + 64, 2 + pr,
                            128 * kc:128 * (kc + 1)],
                        qkT[64 * sub:64 * sub + 64, pr,
                            b * TB + q0:(b + 1) * TB],
                        start=True, stop=True,
                        tile_position=(64 * sub, 0))
                return ps, q0

            def emit_exp(b, pr, kc, ps, q0):
                diag = kc >= 4 * b
                probs = probsp.tile([128, 2, TB], BF16, tag="probs")
                nc.scalar.activation(
                    probs[:, :, q0:], ps[:, :, q0:],
                    mybir.ActivationFunctionType.Exp,
                    scale=1.0 / np.sqrt(HD))
                if diag:
                    for sub in range(2):
                        nc.vector.tensor_mul(
                            probs[:, sub, q0:q0 + 128],
                            probs[:, sub, q0:q0 + 128],
                            tri_sb[:])
                return probs

            def emit_pv(b, pr, kc, probs, q0, pvs):
                nk = 4 * b + 4
                for sub in range(2):
                    h = 2 * pr + sub
                    nc.tensor.matmul(
                        pvs[sub][:, q0:],
                        v_sb[kc][:, h, :],
                        probs[:, sub, q0:],
                        start=(kc == 0), stop=(kc == nk - 1))

            def emit_norm(b, pr, pvs):
                for sub in range(2):
                    den = work.tile([1, TB], F32, tag="den")
                    nc.vector.tensor_copy(den[:], pvs[sub][HD:HD + 1, :])
                    rec = work.tile([1, TB], F32, tag="rec")
                    nc.vector.reciprocal_approx_fast(rec[:], den[:])
                    bc = work.tile([64, TB], F32, tag="bc")
                    nc.gpsimd.partition_broadcast(bc[:], rec[:])
                    nc.vector.tensor_mul(
                        yT[64 * sub:64 * sub + 64, pr,
                           b * TB:(b + 1) * TB],
                        pvs[sub][0:HD, :], bc[:])

            def emit_attn_pair(b, pr):
                """Software-pipelined pass: scores(r+1) and one injected PE
                unit are emitted between exp(r) and PV(r)."""
                nk = 4 * b + 4
                pvs = [ps_pv.tile([HD + 1, TB], F32, tag="pv",
                                  name=f"pv{b}_{pr}_{s}") for s in range(2)]
                ps, q0 = emit_scores(b, pr, 0)
                pend = (0, ps, q0)
                for kc in range(1, nk):
                    pkc, pps, pq0 = pend
                    probs = emit_exp(b, pr, pkc, pps, pq0)
                    ps, q0 = emit_scores(b, pr, kc)
                    inject(1)
                    emit_pv(b, pr, pkc, probs, pq0, pvs)
                    pend = (kc, ps, q0)
                pkc, pps, pq0 = pend
                probs = emit_exp(b, pr, pkc, pps, pq0)
                inject(1)
                emit_pv(b, pr, pkc, probs, pq0, pvs)
                emit_norm(b, pr, pvs)

            # ---------- main schedule ----------
            # QKV(0) runs up front; QKV(b+1)/proj(b) inject into attn(b).
            for u in (unit_qk(0, 0), unit_qk(0, 1), unit_v(0, 0),
                      unit_v(0, 1)):
                u()
            for b in range(NTB):
                if b + 1 < NTB:
                    x_dma(b + 1)
                    pending.extend([unit_qk(b + 1, 0), unit_qk(b + 1, 1),
                                    unit_v(b + 1, 0), unit_v(b + 1, 1)])
                if b > 0:
                    pending.extend([unit_proj(b - 1, tq) for tq in range(4)])
                emit_attn_pair(b, 0)
                emit_attn_pair(b, 1)
            # drain whatever injection didn't absorb, then the final proj
            while pending:
                pending.pop(0)()
            for tq in range(4):
                unit_proj(3, tq)()
    nc.compile()
    return nc


def _tri_np():
    # tri[k, q] = 1 where k <= q (block-local causal keep mask)
    kr = np.arange(128)[:, None]
    qc = np.arange(128)[None, :]
    return (kr <= qc).astype(ml_dtypes.bfloat16)


def _swizzle(a):
    """[R, C] with R = 128*n -> [128, n, C] partition-major."""
    r, c = a.shape
    return np.ascontiguousarray(
        a.reshape(r // 128, 128, c).transpose(1, 0, 2))


def _prep_in_maps(x, w_qkv, w_proj):
    bf = ml_dtypes.bfloat16
    tri = _tri_np()
    in_maps = []
    for c in range(NCORES):
        b, g = c // 4, c % 4
        heads = slice(g * HPC * HD, (g + 1) * HPC * HD)      # 256 rows
        wq = w_qkv[0 * D:1 * D][heads]                        # [256, 1024]
        wk = w_qkv[1 * D:2 * D][heads]
        wvm = w_qkv[2 * D:3 * D][heads]
        xT = np.ascontiguousarray(x[b].T).astype(bf)          # [1024, 2048]
        # xs[b] = [128, DCH, TB] slice of the swizzled xT
        xsw = _swizzle(xT)                                    # [128, 8, 2048]
        xs = np.ascontiguousarray(
            xsw.reshape(128, DCH, NTB, TB).transpose(2, 0, 1, 3)
            .reshape(NTB * 128, DCH, TB))
        in_maps.append({
            "xs": xs,                                         # [512, 8, 512]
            "wqk": _swizzle(
                np.concatenate([wq, wk], axis=0).T.astype(bf)),
            "wv": _swizzle(wvm.T.astype(bf)),                 # [128, 8, 256]
            "wp": _swizzle(w_proj[:, heads].T.astype(bf)),    # [128, 2,1024]
            "tri": tri,
        })
    return in_maps


def kernel(x, w_qkv, w_proj, _trace=False):
    x = np.asarray(x, dtype=np.float32)
    w_qkv = np.asarray(w_qkv, dtype=np.float32)
    w_proj = np.asarray(w_proj, dtype=np.float32)
    if _trace:
        _ensure_ntff_hook()
    if "nc" not in _cache:
        _cache["nc"] = _build_nc()
    nc = _cache["nc"]
    in_maps = _prep_in_maps(x, w_qkv, w_proj)
    res = run_bass_kernel_spmd(nc, in_maps, list(range(NCORES)),
                               trace=_trace)
    out = np.zeros((B, T, D), dtype=np.float32)
    for c in range(NCORES):
        out[c // 4] += res.results[c]["out"]
    if _trace:
        _cache["last_result"] = res
    return out


# revision 20
# speedup vs baseline: 1.0147x; 1.0147x over previous
"""Causal self-attention Trainium2 kernel (8 NeuronCores).

Sharding: tensor-parallel over heads x data-parallel over batch.
Core c handles batch b = c // 4 and head group g = c % 4 (4 heads of 16).
Each core computes q/k/v projections for its heads, causal attention, and a
partial output projection (its 256 columns of the 1024-wide contraction);
the host sums the 4 partials per batch.

v3 design (bf16, PE row-tiled scores, software-pipelined + work injection):
  - All matmul inputs bf16 (PSUM stays fp32). PE streams 1 col/cycle@2.4GHz.
  - Scores for a head pair are row-tiled: both heads' hd-dims occupy
    disjoint 64-partition bands of one qkT chunk, so the two K=64 matmuls
    share one moving stream and run concurrently (explicit tile_position).
  - Attention per head-pair pass: one 2-bank PSUM slot per round, one
    batched exp ACTIVATE over both heads ((N+352)cyc amortized), causal
    column trim on diagonal chunks, [128,128] triangle mask on DVE.
  - PV uses the ones-column denominator trick (M=65); normalization is
    fused into the PV drain (psum * bcast(1/den) -> bf16 yT).
  - QKV(b+1) and proj(b) are split into ~1us PE units and injected into
    the attention rounds (one unit after each exp emission) so the PE
    works under the ACT/PE-round shadow instead of serializing phases.
  - Host pre-swizzles inputs to [128, ...] partition-major layouts so all
    input DMAs move 4-8KB contiguous lines per partition.
"""

import numpy as np
import ml_dtypes

import concourse.bass as bass
from concourse import bacc
import concourse.mybir as mybir
import concourse.tile as tile
from concourse.bass_utils import run_bass_kernel_spmd

B, T, D, H = 2, 2048, 1024, 16
HD = D // H          # 64
HPC = 4              # heads per core
NCORES = 8
EQK = 2 * HPC * HD   # 512 rows of q+k per core
EV = HPC * HD        # 256 rows of v per core
TB = 512             # t/q block
NTB = T // TB        # 4
TC = 128             # t chunk
NTC = T // TC        # 16
DCH = D // 128       # 8 contraction chunks
F32 = mybir.dt.float32
BF16 = mybir.dt.bfloat16

_cache = {}


def _ensure_ntff_hook():
    """The agent image's antenv lacks axon_hooks; fabricate it so
    run_bass_kernel_spmd(trace=True) can capture NTFF profiles."""
    import sys
    import types
    try:
        import antenv.axon_hooks  # noqa: F401
        return
    except ImportError:
        pass
    try:
        import antenv
        from trn_agent_boot.trn_boot import _ntff_profile_via_ctypes
        hook = {"h": _ntff_profile_via_ctypes("/opt/axon/libaxon_pjrt.so")}
        m = types.ModuleType("antenv.axon_hooks")
        m.get_axon_ntff_profile_hook = lambda: hook["h"]
        m.set_axon_ntff_profile_hook = lambda h: hook.update(h=h)
        sys.modules["antenv.axon_hooks"] = m
        antenv.axon_hooks = m
    except Exception:
        pass


def _build_nc():
    nc = bacc.Bacc("TRN2", target_bir_lowering=False, debug=False,
                  num_devices=NCORES)
    # host-swizzled partition-major layouts (fat contiguous DMA lines)
    xs = nc.dram_tensor("xs", [NTB * 128, DCH, TB], BF16, kind="ExternalInput")
    wqka = nc.dram_tensor("wqka", [128, DCH, EQK // 2], BF16,
                          kind="ExternalInput")
    wqkb = nc.dram_tensor("wqkb", [128, DCH, EQK // 2], BF16,
                          kind="ExternalInput")
    wv = nc.dram_tensor("wv", [128, DCH, EV], BF16, kind="ExternalInput")
    wp = nc.dram_tensor("wp", [128, 2, D], BF16, kind="ExternalInput")
    tri = nc.dram_tensor("tri", [128, 128], BF16, kind="ExternalInput")
    out = nc.dram_tensor("out", [T, D], F32, kind="ExternalOutput")

    with tile.TileContext(nc) as tc:
        with (
            nc.allow_low_precision(reason="bf16 matmul inputs; psum stays fp32"),
            tc.tile_pool(name="persist", bufs=1) as persist,
            tc.tile_pool(name="xin", bufs=2) as xin,
            tc.tile_pool(name="work", bufs=4) as work,
            tc.tile_pool(name="probsp", bufs=4) as probsp,
            tc.tile_pool(name="outp", bufs=3) as outp,
            tc.tile_pool(name="ps_mm", bufs=2, space="PSUM") as ps_mm,
            tc.tile_pool(name="ps_pv", bufs=4, space="PSUM") as ps_pv,
        ):
            # ---- input DMAs: x(0) first (QKV(0) is the critical path) ----
            x_tiles = {}
            x_tiles[0] = xin.tile([128, DCH, TB], BF16, tag="x", name="x_t0")
            nc.sync.dma_start(x_tiles[0][:], xs[0:128, :, :])
            wqk_a = persist.tile([128, DCH, EQK // 2], BF16)  # 4KB/part
            nc.sync.dma_start(wqk_a[:], wqka[:, :, :])
            wqk_b = persist.tile([128, DCH, EQK // 2], BF16)  # 4KB/part
            nc.sync.dma_start(wqk_b[:], wqkb[:, :, :])
            wqk_ab = (wqk_a, wqk_b)
            wv_sb = persist.tile([128, DCH, EV], BF16)     # 4KB/part
            nc.sync.dma_start(wv_sb[:], wv[:, :, :])
            wp_sb = persist.tile([128, 2, D], BF16)        # 4KB/part
            nc.sync.dma_start(wp_sb[:], wp[:, :, :])
            tri_sb = persist.tile([128, 128], BF16)
            nc.sync.dma_start(tri_sb[:], tri[:, :])

            # qkT[p, c, t]: c in {0: q h01, 1: q h23, 2: k h01, 3: k h23};
            # head pair member at base partition 0/64.
            qkT = persist.tile([128, 4, T], BF16, name="qkT")    # 16KB/part
            # v_sb[t_chunk]: [128, h, 65]; col 64 of each head slot is 1.0
            v_sb = [persist.tile([128, HPC, HD + 1], BF16, tag=f"v{i}",
                                name=f"v{i}")
                    for i in range(NTC)]
            # yT: normalized attention output, [p, c, t]; c=0 heads01, 1 h23
            yT = persist.tile([128, 2, T], BF16, name="yT")      # 8KB/part

            # ones columns for the denominator trick
            for i in range(NTC):
                nc.gpsimd.memset(v_sb[i][:, :, HD], 1.0)

            # ---------- PE work units (for injection) ----------
            def x_dma(b):
                x_tiles[b] = xin.tile([128, DCH, TB], BF16, tag="x",
                                      name=f"x_t{b}")
                nc.sync.dma_start(x_tiles[b][:],
                                  xs[128 * b:128 * (b + 1), :, :])

            def unit_qk(b, pr):
                def go():
                    x_t = x_tiles[b]
                    ps = ps_mm.tile([128, 2, TB], F32, tag="mm", name="ps_qk")
                    for sub in range(2):
                        for dc in range(DCH):
                            nc.tensor.matmul(
                                ps[:, sub, :],
                                wqk_ab[pr][:, dc, 128 * sub:128 * (sub + 1)],
                                x_t[:, dc, :],
                                start=(dc == 0), stop=(dc == DCH - 1))
                    nc.vector.tensor_copy(
                        qkT[:, 2 * pr:2 * pr + 2, b * TB:(b + 1) * TB], ps[:])
                return go

            def unit_v(b, pr):
                def go():
                    x_t = x_tiles[b]
                    ps = ps_mm.tile([128, 2, TB], F32, tag="mm", name="ps_v")
                    for sub in range(2):
                        t2 = 2 * pr + sub
                        for dc in range(DCH):
                            nc.tensor.matmul(
                                ps[:, sub, 0:EV],
                                x_t[:, dc, 128 * t2:128 * (t2 + 1)],
                                wv_sb[:, dc, :],
                                start=(dc == 0), stop=(dc == DCH - 1))
                    for sub in range(2):
                        tc_i = 4 * b + 2 * pr + sub
                        nc.vector.tensor_copy(
                            v_sb[tc_i][:, :, 0:HD],
                            ps[:, sub, 0:EV].rearrange("p (h f) -> p h f",
                                                       h=HPC))
                return go

            def unit_proj(b, tq, tail=False):
                def go():
                    tc_i = 4 * b + tq
                    ps = ps_mm.tile([128, 2, TB], F32, tag="mm", name="ps_pj")
                    for e in range(2):
                        for c in range(2):
                            nc.tensor.matmul(
                                ps[:, e, :],
                                yT[:, c, 128 * tc_i:128 * (tc_i + 1)],
                                wp_sb[:, c, 512 * e:512 * (e + 1)],
                                start=(c == 0), stop=(c == 1))
                    o_sb = outp.tile([128, 2, TB], F32, tag="o")
                    if tail:
                        # ACT is idle at the kernel tail; keep DVE clear
                        nc.scalar.copy(o_sb[:], ps[:])
                    else:
                        nc.vector.tensor_copy(o_sb[:], ps[:])
                    nc.sync.dma_start(
                        out[128 * tc_i:128 * (tc_i + 1), :],
                        o_sb.rearrange("p a b -> p (a b)"))
                return go

            pending = []

            def inject(k=1):
                for _ in range(k):
                    if pending:
                        pending.pop(0)()

            # ---------- attention ----------
            def emit_scores(b, pr, kc):
                j = kc - 4 * b
                q0 = 128 * j if j >= 0 else 0
                ps = ps_mm.tile([128, 2, TB], F32, tag="mm", name="ps_s")
                for sub in range(2):
                    nc.tensor.matmul(
                        ps[:, sub, q0:],
                        qkT[64 * sub:64 * sub + 64, 2 + pr,
                            128 * kc:128 * (kc + 1)],
                        qkT[64 * sub:64 * sub + 64, pr,
                            b * TB + q0:(b + 1) * TB],
                        start=True, stop=True,
                        tile_position=(64 * sub, 0))
                return ps, q0

            def emit_exp(b, pr, kc, ps, q0):
                diag = kc >= 4 * b
                probs = probsp.tile([128, 2, TB], BF16, tag="probs")
                nc.scalar.activation(
                    probs[:, :, q0:], ps[:, :, q0:],
                    mybir.ActivationFunctionType.Exp,
                    scale=1.0 / np.sqrt(HD))
                if diag:
                    for sub in range(2):
                        nc.vector.tensor_mul(
                            probs[:, sub, q0:q0 + 128],
                            probs[:, sub, q0:q0 + 128],
                            tri_sb[:])
                return probs

            def emit_pv(b, pr, kc, probs, q0, pvs):
                nk = 4 * b + 4
                for sub in range(2):
                    h = 2 * pr + sub
                    nc.tensor.matmul(
                        pvs[sub][:, q0:],
                        v_sb[kc][:, h, :],
                        probs[:, sub, q0:],
                        start=(kc == 0), stop=(kc == nk - 1))

            def emit_norm(b, pr, pvs):
                for sub in range(2):
                    den = work.tile([1, TB], F32, tag="den")
                    nc.vector.tensor_copy(den[:], pvs[sub][HD:HD + 1, :])
                    rec = work.tile([1, TB], F32, tag="rec")
                    nc.vector.reciprocal_approx_fast(rec[:], den[:])
                    bc = work.tile([64, TB], F32, tag="bc")
                    nc.gpsimd.partition_broadcast(bc[:], rec[:])
                    nc.vector.tensor_mul(
                        yT[64 * sub:64 * sub + 64, pr,
                           b * TB:(b + 1) * TB],
                        pvs[sub][0:HD, :], bc[:])
                    inject(1)

            def emit_attn_pair(b, pr):
                """Software-pipelined pass: scores(r+1) and one injected PE
                unit are emitted between exp(r) and PV(r)."""
                nk = 4 * b + 4
                pvs = [ps_pv.tile([HD + 1, TB], F32, tag="pv",
                                  name=f"pv{b}_{pr}_{s}") for s in range(2)]
                ps, q0 = emit_scores(b, pr, 0)
                pend = (0, ps, q0)
                for kc in range(1, nk):
                    pkc, pps, pq0 = pend
                    probs = emit_exp(b, pr, pkc, pps, pq0)
                    ps, q0 = emit_scores(b, pr, kc)
                    inject(1)
                    emit_pv(b, pr, pkc, probs, pq0, pvs)
                    pend = (kc, ps, q0)
                pkc, pps, pq0 = pend
                probs = emit_exp(b, pr, pkc, pps, pq0)
                inject(1)
                emit_pv(b, pr, pkc, probs, pq0, pvs)
                emit_norm(b, pr, pvs)

            # ---------- main schedule ----------
            # QKV(0) runs up front; QKV(b+1)/proj(b) inject into attn(b).
            for u in (unit_qk(0, 0), unit_qk(0, 1), unit_v(0, 0),
                      unit_v(0, 1)):
                u()
            for b in range(NTB):
                if b + 1 < NTB:
                    x_dma(b + 1)
                    pending.extend([unit_qk(b + 1, 0), unit_qk(b + 1, 1),
                                    unit_v(b + 1, 0), unit_v(b + 1, 1)])
                if b > 0:
                    pending.extend([unit_proj(b - 1, tq) for tq in range(4)])
                emit_attn_pair(b, 0)
                emit_attn_pair(b, 1)
            # drain whatever injection didn't absorb, then the final proj
            while pending:
                pending.pop(0)()
            for tq in range(4):
                unit_proj(3, tq, tail=True)()
    nc.compile()
    return nc


def _tri_np():
    # tri[k, q] = 1 where k <= q (block-local causal keep mask)
    kr = np.arange(128)[:, None]
    qc = np.arange(128)[None, :]
    return (kr <= qc).astype(ml_dtypes.bfloat16)


def _swizzle(a):
    """[R, C] with R = 128*n -> [128, n, C] partition-major."""
    r, c = a.shape
    return np.ascontiguousarray(
        a.reshape(r // 128, 128, c).transpose(1, 0, 2))


def _prep_in_maps(x, w_qkv, w_proj):
    bf = ml_dtypes.bfloat16
    tri = _tri_np()
    in_maps = []
    for c in range(NCORES):
        b, g = c // 4, c % 4
        heads = slice(g * HPC * HD, (g + 1) * HPC * HD)      # 256 rows
        wq = w_qkv[0 * D:1 * D][heads]                        # [256, 1024]
        wk = w_qkv[1 * D:2 * D][heads]
        wvm = w_qkv[2 * D:3 * D][heads]
        xT = np.ascontiguousarray(x[b].T).astype(bf)          # [1024, 2048]
        # xs[b] = [128, DCH, TB] slice of the swizzled xT
        xsw = _swizzle(xT)                                    # [128, 8, 2048]
        xs = np.ascontiguousarray(
            xsw.reshape(128, DCH, NTB, TB).transpose(2, 0, 1, 3)
            .reshape(NTB * 128, DCH, TB))
        wqks = _swizzle(
            np.concatenate([wq, wk], axis=0).T.astype(bf))    # [128, 8, 512]
        in_maps.append({
            "xs": xs,                                         # [512, 8, 512]
            "wqka": np.ascontiguousarray(wqks[:, :, 0:EQK // 2]),
            "wqkb": np.ascontiguousarray(wqks[:, :, EQK // 2:]),
            "wv": _swizzle(wvm.T.astype(bf)),                 # [128, 8, 256]
            "wp": _swizzle(w_proj[:, heads].T.astype(bf)),    # [128, 2,1024]
            "tri": tri,
        })
    return in_maps


def kernel(x, w_qkv, w_proj, _trace=False):
    x = np.asarray(x, dtype=np.float32)
    w_qkv = np.asarray(w_qkv, dtype=np.float32)
    w_proj = np.asarray(w_proj, dtype=np.float32)
    if _trace:
        _ensure_ntff_hook()
    if "nc" not in _cache:
        _cache["nc"] = _build_nc()
    nc = _cache["nc"]
    in_maps = _prep_in_maps(x, w_qkv, w_proj)
    res = run_bass_kernel_spmd(nc, in_maps, list(range(NCORES)),
                               trace=_trace)
    out = np.zeros((B, T, D), dtype=np.float32)
    for c in range(NCORES):
        out[c // 4] += res.results[c]["out"]
    if _trace:
        _cache["last_result"] = res
    return out


# revision 22
# speedup vs baseline: 1.2358x; 1.2179x over previous
"""Causal self-attention Trainium2 kernel (8 NeuronCores).

Sharding: tensor-parallel over heads x data-parallel over batch.
Core c handles batch b = c // 4 and head group g = c % 4 (4 heads of 16).
Each core computes q/k/v projections for its heads, causal attention, and a
partial output projection (its 256 columns of the 1024-wide contraction);
the host sums the 4 partials per batch.

v3 design (bf16, PE row-tiled scores, software-pipelined + work injection):
  - All matmul inputs bf16 (PSUM stays fp32). PE streams 1 col/cycle@2.4GHz.
  - Scores for a head pair are row-tiled: both heads' hd-dims occupy
    disjoint 64-partition bands of one qkT chunk, so the two K=64 matmuls
    share one moving stream and run concurrently (explicit tile_position).
  - Attention per head-pair pass: one 2-bank PSUM slot per round, one
    batched exp ACTIVATE over both heads ((N+352)cyc amortized), causal
    column trim on diagonal chunks, [128,128] triangle mask on DVE.
  - PV uses the ones-column denominator trick (M=65); normalization is
    fused into the PV drain (psum * bcast(1/den) -> bf16 yT).
  - QKV(b+1) and proj(b) are split into ~1us PE units and injected into
    the attention rounds (one unit after each exp emission) so the PE
    works under the ACT/PE-round shadow instead of serializing phases.
  - Host pre-swizzles inputs to [128, ...] partition-major layouts so all
    input DMAs move 4-8KB contiguous lines per partition.
"""

import numpy as np
import ml_dtypes

import concourse.bass as bass
from concourse import bacc
import concourse.mybir as mybir
import concourse.tile as tile
from concourse.bass_utils import run_bass_kernel_spmd

B, T, D, H = 2, 2048, 1024, 16
HD = D // H          # 64
HPC = 4              # heads per core
NCORES = 8
EQK = 2 * HPC * HD   # 512 rows of q+k per core
EV = HPC * HD        # 256 rows of v per core
TB = 512             # t/q block
NTB = T // TB        # 4
TC = 128             # t chunk
NTC = T // TC        # 16
DCH = D // 128       # 8 contraction chunks
F32 = mybir.dt.float32
BF16 = mybir.dt.bfloat16

_cache = {}


def _ensure_ntff_hook():
    """The agent image's antenv lacks axon_hooks; fabricate it so
    run_bass_kernel_spmd(trace=True) can capture NTFF profiles."""
    import sys
    import types
    try:
        import antenv.axon_hooks  # noqa: F401
        return
    except ImportError:
        pass
    try:
        import antenv
        from trn_agent_boot.trn_boot import _ntff_profile_via_ctypes
        hook = {"h": _ntff_profile_via_ctypes("/opt/axon/libaxon_pjrt.so")}
        m = types.ModuleType("antenv.axon_hooks")
        m.get_axon_ntff_profile_hook = lambda: hook["h"]
        m.set_axon_ntff_profile_hook = lambda h: hook.update(h=h)
        sys.modules["antenv.axon_hooks"] = m
        antenv.axon_hooks = m
    except Exception:
        pass


def _build_nc():
    nc = bacc.Bacc("TRN2", target_bir_lowering=False, debug=False,
                  num_devices=NCORES)
    # host-swizzled partition-major layouts (fat contiguous DMA lines)
    xs = nc.dram_tensor("xs", [NTB * 128, DCH, TB], BF16, kind="ExternalInput")
    wqka = nc.dram_tensor("wqka", [128, DCH, EQK // 2], BF16,
                          kind="ExternalInput")
    wqkb = nc.dram_tensor("wqkb", [128, DCH, EQK // 2], BF16,
                          kind="ExternalInput")
    wv = nc.dram_tensor("wv", [128, DCH, EV], BF16, kind="ExternalInput")
    wp = nc.dram_tensor("wp", [128, 2, D], BF16, kind="ExternalInput")
    tri = nc.dram_tensor("tri", [128, 128], BF16, kind="ExternalInput")
    out = nc.dram_tensor("out", [T, D], F32, kind="ExternalOutput")

    with tile.TileContext(nc) as tc:
        with (
            nc.allow_low_precision(reason="bf16 matmul inputs; psum stays fp32"),
            tc.tile_pool(name="persist", bufs=1) as persist,
            tc.tile_pool(name="xin", bufs=2) as xin,
            tc.tile_pool(name="work", bufs=4) as work,
            tc.tile_pool(name="probsp", bufs=4) as probsp,
            tc.tile_pool(name="outp", bufs=3) as outp,
            tc.tile_pool(name="ps_mm", bufs=2, space="PSUM") as ps_mm,
            tc.tile_pool(name="ps_pv", bufs=4, space="PSUM") as ps_pv,
        ):
            # ---- input DMAs: x(0) first (QKV(0) is the critical path) ----
            x_tiles = {}
            x_tiles[0] = xin.tile([128, DCH, TB], BF16, tag="x", name="x_t0")
            # split b=0's x load so the first q/k chain (dc order) can start
            # after ~1/4 of it has landed
            for q in range(4):
                nc.sync.dma_start(x_tiles[0][:, 2 * q:2 * q + 2, :],
                                  xs[0:128, 2 * q:2 * q + 2, :])
            wqk_a = persist.tile([128, DCH, EQK // 2], BF16)  # 4KB/part
            nc.sync.dma_start(wqk_a[:], wqka[:, :, :])
            wqk_b = persist.tile([128, DCH, EQK // 2], BF16)  # 4KB/part
            nc.sync.dma_start(wqk_b[:], wqkb[:, :, :])
            wqk_ab = (wqk_a, wqk_b)
            wv_sb = persist.tile([128, DCH, EV], BF16)     # 4KB/part
            nc.sync.dma_start(wv_sb[:], wv[:, :, :])
            wp_sb = persist.tile([128, 2, D], BF16)        # 4KB/part
            nc.sync.dma_start(wp_sb[:], wp[:, :, :])
            tri_sb = persist.tile([128, 128], BF16)
            nc.sync.dma_start(tri_sb[:], tri[:, :])

            # qkT[p, c, t]: c in {0: q h01, 1: q h23, 2: k h01, 3: k h23};
            # head pair member at base partition 0/64.
            qkT = persist.tile([128, 4, T], BF16, name="qkT")    # 16KB/part
            # v_sb[t_chunk]: [128, h, 65]; col 64 of each head slot is 1.0
            v_sb = [persist.tile([128, HPC, HD + 1], BF16, tag=f"v{i}",
                                name=f"v{i}")
                    for i in range(NTC)]
            # yT: normalized attention output, [p, c, t]; c=0 heads01, 1 h23
            yT = persist.tile([128, 2, T], BF16, name="yT")      # 8KB/part

            # ones columns for the denominator trick
            for i in range(NTC):
                nc.gpsimd.memset(v_sb[i][:, :, HD], 1.0)

            # ---------- PE work units (for injection) ----------
            def x_dma(b):
                x_tiles[b] = xin.tile([128, DCH, TB], BF16, tag="x",
                                      name=f"x_t{b}")
                nc.sync.dma_start(x_tiles[b][:],
                                  xs[128 * b:128 * (b + 1), :, :])

            def unit_qk(b, pr):
                def go():
                    x_t = x_tiles[b]
                    ps = ps_mm.tile([128, 2, TB], F32, tag="mm", name="ps_qk")
                    for sub in range(2):
                        for dc in range(DCH):
                            nc.tensor.matmul(
                                ps[:, sub, :],
                                wqk_ab[pr][:, dc, 128 * sub:128 * (sub + 1)],
                                x_t[:, dc, :],
                                start=(dc == 0), stop=(dc == DCH - 1))
                    nc.vector.tensor_copy(
                        qkT[:, 2 * pr:2 * pr + 2, b * TB:(b + 1) * TB], ps[:])
                return go

            def unit_v(b, pr):
                def go():
                    x_t = x_tiles[b]
                    ps = ps_mm.tile([128, 2, TB], F32, tag="mm", name="ps_v")
                    for sub in range(2):
                        t2 = 2 * pr + sub
                        for dc in range(DCH):
                            nc.tensor.matmul(
                                ps[:, sub, 0:EV],
                                x_t[:, dc, 128 * t2:128 * (t2 + 1)],
                                wv_sb[:, dc, :],
                                start=(dc == 0), stop=(dc == DCH - 1))
                    for sub in range(2):
                        tc_i = 4 * b + 2 * pr + sub
                        nc.vector.tensor_copy(
                            v_sb[tc_i][:, :, 0:HD],
                            ps[:, sub, 0:EV].rearrange("p (h f) -> p h f",
                                                       h=HPC))
                return go

            def unit_proj(b, tq, tail=False):
                def go():
                    tc_i = 4 * b + tq
                    ps = ps_mm.tile([128, 2, TB], F32, tag="mm", name="ps_pj")
                    for e in range(2):
                        for c in range(2):
                            nc.tensor.matmul(
                                ps[:, e, :],
                                yT[:, c, 128 * tc_i:128 * (tc_i + 1)],
                                wp_sb[:, c, 512 * e:512 * (e + 1)],
                                start=(c == 0), stop=(c == 1))
                    o_sb = outp.tile([128, 2, TB], F32, tag="o")
                    if tail:
                        # ACT is idle at the kernel tail; keep DVE clear
                        nc.scalar.copy(o_sb[:], ps[:])
                    else:
                        nc.vector.tensor_copy(o_sb[:], ps[:])
                    nc.sync.dma_start(
                        out[128 * tc_i:128 * (tc_i + 1), :],
                        o_sb.rearrange("p a b -> p (a b)"))
                return go

            pending = []

            def inject(k=1):
                for _ in range(k):
                    if pending:
                        pending.pop(0)()

            # ---------- attention ----------
            def emit_scores(b, pr, kc):
                j = kc - 4 * b
                q0 = 128 * j if j >= 0 else 0
                ps = ps_mm.tile([128, 2, TB], F32, tag="mm", name="ps_s")
                for sub in range(2):
                    nc.tensor.matmul(
                        ps[:, sub, q0:],
                        qkT[64 * sub:64 * sub + 64, 2 + pr,
                            128 * kc:128 * (kc + 1)],
                        qkT[64 * sub:64 * sub + 64, pr,
                            b * TB + q0:(b + 1) * TB],
                        start=True, stop=True,
                        tile_position=(64 * sub, 0))
                return ps, q0

            def emit_exp(b, pr, kc, ps, q0):
                diag = kc >= 4 * b
                probs = probsp.tile([128, 2, TB], BF16, tag="probs")
                nc.scalar.activation(
                    probs[:, :, q0:], ps[:, :, q0:],
                    mybir.ActivationFunctionType.Exp,
                    scale=1.0 / np.sqrt(HD))
                if diag:
                    for sub in range(2):
                        nc.vector.tensor_mul(
                            probs[:, sub, q0:q0 + 128],
                            probs[:, sub, q0:q0 + 128],
                            tri_sb[:])
                return probs

            def emit_pv(b, pr, kc, probs, q0, pvs):
                nk = 4 * b + 4
                for sub in range(2):
                    h = 2 * pr + sub
                    nc.tensor.matmul(
                        pvs[sub][:, q0:],
                        v_sb[kc][:, h, :],
                        probs[:, sub, q0:],
                        start=(kc == 0), stop=(kc == nk - 1))

            def emit_norm(b, pr, pvs):
                for sub in range(2):
                    den = work.tile([1, TB], F32, tag="den")
                    nc.vector.tensor_copy(den[:], pvs[sub][HD:HD + 1, :])
                    rec = work.tile([1, TB], F32, tag="rec")
                    nc.vector.reciprocal_approx_fast(rec[:], den[:])
                    bc = work.tile([64, TB], F32, tag="bc")
                    nc.gpsimd.partition_broadcast(bc[:], rec[:])
                    nc.vector.tensor_mul(
                        yT[64 * sub:64 * sub + 64, pr,
                           b * TB:(b + 1) * TB],
                        pvs[sub][0:HD, :], bc[:])
                    inject(1)

            def emit_attn_pair(b, pr):
                """Software-pipelined pass: scores(r+1) and one injected PE
                unit are emitted between exp(r) and PV(r)."""
                nk = 4 * b + 4
                pvs = [ps_pv.tile([HD + 1, TB], F32, tag="pv",
                                  name=f"pv{b}_{pr}_{s}") for s in range(2)]
                ps, q0 = emit_scores(b, pr, 0)
                pend = (0, ps, q0)
                for kc in range(1, nk):
                    pkc, pps, pq0 = pend
                    probs = emit_exp(b, pr, pkc, pps, pq0)
                    ps, q0 = emit_scores(b, pr, kc)
                    inject(1)
                    emit_pv(b, pr, pkc, probs, pq0, pvs)
                    pend = (kc, ps, q0)
                pkc, pps, pq0 = pend
                probs = emit_exp(b, pr, pkc, pps, pq0)
                inject(1)
                emit_pv(b, pr, pkc, probs, pq0, pvs)
                emit_norm(b, pr, pvs)

            # ---------- main schedule ----------
            # QKV(0) runs up front; QKV(b+1)/proj(b) inject into attn(b).
            for u in (unit_qk(0, 0), unit_qk(0, 1), unit_v(0, 0),
                      unit_v(0, 1)):
                u()
            for b in range(NTB):
                # proj units inject into attention round slack (small, fits
                # under the ACT cadence); QKV(b+1) runs serially at the
                # block boundary at full PE clock.
                if b > 0:
                    pending.extend([unit_proj(b - 1, tq) for tq in range(4)])
                if b + 1 < NTB:
                    x_dma(b + 1)
                emit_attn_pair(b, 0)
                emit_attn_pair(b, 1)
                if b + 1 < NTB:
                    for u in (unit_qk(b + 1, 0), unit_qk(b + 1, 1),
                              unit_v(b + 1, 0), unit_v(b + 1, 1)):
                        u()
            # drain whatever injection didn't absorb, then the final proj
            while pending:
                pending.pop(0)()
            for tq in range(4):
                unit_proj(3, tq, tail=True)()
    nc.compile()
    return nc


def _tri_np():
    # tri[k, q] = 1 where k <= q (block-local causal keep mask)
    kr = np.arange(128)[:, None]
    qc = np.arange(128)[None, :]
    return (kr <= qc).astype(ml_dtypes.bfloat16)


def _swizzle(a):
    """[R, C] with R = 128*n -> [128, n, C] partition-major."""
    r, c = a.shape
    return np.ascontiguousarray(
        a.reshape(r // 128, 128, c).transpose(1, 0, 2))


def _prep_in_maps(x, w_qkv, w_proj):
    bf = ml_dtypes.bfloat16
    tri = _tri_np()
    in_maps = []
    for c in range(NCORES):
        b, g = c // 4, c % 4
        heads = slice(g * HPC * HD, (g + 1) * HPC * HD)      # 256 rows
        wq = w_qkv[0 * D:1 * D][heads]                        # [256, 1024]
        wk = w_qkv[1 * D:2 * D][heads]
        wvm = w_qkv[2 * D:3 * D][heads]
        xT = np.ascontiguousarray(x[b].T).astype(bf)          # [1024, 2048]
        # xs[b] = [128, DCH, TB] slice of the swizzled xT
        xsw = _swizzle(xT)                                    # [128, 8, 2048]
        xs = np.ascontiguousarray(
            xsw.reshape(128, DCH, NTB, TB).transpose(2, 0, 1, 3)
            .reshape(NTB * 128, DCH, TB))
        wqks = _swizzle(
            np.concatenate([wq, wk], axis=0).T.astype(bf))    # [128, 8, 512]
        in_maps.append({
            "xs": xs,                                         # [512, 8, 512]
            "wqka": np.ascontiguousarray(wqks[:, :, 0:EQK // 2]),
            "wqkb": np.ascontiguousarray(wqks[:, :, EQK // 2:]),
            "wv": _swizzle(wvm.T.astype(bf)),                 # [128, 8, 256]
            "wp": _swizzle(w_proj[:, heads].T.astype(bf)),    # [128, 2,1024]
            "tri": tri,
        })
    return in_maps


def kernel(x, w_qkv, w_proj, _trace=False):
    x = np.asarray(x, dtype=np.float32)
    w_qkv = np.asarray(w_qkv, dtype=np.float32)
    w_proj = np.asarray(w_proj, dtype=np.float32)
    if _trace:
        _ensure_ntff_hook()
    if "nc" not in _cache:
        _cache["nc"] = _build_nc()
    nc = _cache["nc"]
    in_maps = _prep_in_maps(x, w_qkv, w_proj)
    res = run_bass_kernel_spmd(nc, in_maps, list(range(NCORES)),
                               trace=_trace)
    out = np.zeros((B, T, D), dtype=np.float32)
    for c in range(NCORES):
        out[c // 4] += res.results[c]["out"]
    if _trace:
        _cache["last_result"] = res
    return out
